# revision 13
# baseline (speedup 1.0000x reference)
"""Causal self-attention kernel for Trainium2, 8 NeuronCores.

Sharding: core j handles batch j//4 and heads 4*(j%4) .. 4*(j%4)+3
(tensor-parallel over heads within a batch replica group of 4 cores).

Per-core on-device pipeline (all matmuls bf16, fp32 accumulate):
  1. qkv^T = W^T x^T  (feature-major: Q^T/K^T/V^T [128=2 heads, T])
  2. V^T -> V token-major via xbar DMA transposes; ones column appended
     so the AV matmul also produces softmax row-sums.
  3. S^T[k,q] = (K^T)^T-stationary matmul vs Q^T (per 128-k-block), causal
     block-skipped; exp on ScalarE (no max subtraction needed: logits are
     O(0.1) by construction of the 0.1/sqrt(D) scale); diagonal 128x128
     blocks masked by a triangular multiply.
  4. y^T[d,q] (+ row-sums) = [V|1]-stationary matmul vs P^T, accumulated
     over k-blocks; normalized by 1/rowsum (gpsimd partition-broadcast).
  5. partial = y^T.T @ W_proj[rows of own heads]  -> [T, C] fp32.
Host sums the 4 partials per batch and adds b_proj (the tensor-parallel
unshard step).
"""

import sys

if "/opt/trn_rl_repo" not in sys.path:
    sys.path.insert(0, "/opt/trn_rl_repo")

import numpy as np
import ml_dtypes

B, T, C, H, D = 2, 2048, 1024, 16, 64
SCALE = 0.1 / (D**0.5)
HPC = 4          # heads per core
PAIRS = 2        # head pairs per core (2 heads of 64 feats -> 128 partitions)
FEAT = 3 * HPC * D  # 768 qkv features per core
NCORES = 8

_CACHE = {}


def build_nc(t=T):
    import concourse.mybir as mybir
    import concourse.tile as tile
    from concourse import bacc
    from contextlib import ExitStack

    f32 = mybir.dt.float32
    bf16 = mybir.dt.bfloat16
    Exp = mybir.ActivationFunctionType.Exp

    kblks = t // 128   # 128-wide key blocks per sequence
    qch = t // 512     # 512-wide query chunks per sequence

    nc = bacc.Bacc("TRN2")
    xt = nc.declare_dram_parameter("xt", [C, t], bf16, isOutput=False)
    wqkv = nc.declare_dram_parameter("wqkv", [C, FEAT], bf16, isOutput=False)
    bqkv = nc.declare_dram_parameter("bqkv", [FEAT // 128, 128, 1], f32, isOutput=False)
    wproj = nc.declare_dram_parameter("wproj", [HPC * D, C], bf16, isOutput=False)
    trimask = nc.declare_dram_parameter("trimask", [128, 128], bf16, isOutput=False)
    partial = nc.declare_dram_parameter("partial", [t, C], f32, isOutput=True)

    with tile.TileContext(nc) as tc, ExitStack() as ctx:
        persist = ctx.enter_context(tc.tile_pool(name="persist", bufs=1))
        psum_s = ctx.enter_context(tc.tile_pool(name="psum_s", bufs=5, space="PSUM"))
        psum_y = ctx.enter_context(tc.tile_pool(name="psum_y", bufs=3, space="PSUM"))
        pt_pool = ctx.enter_context(tc.tile_pool(name="pt_pool", bufs=6))
        misc = ctx.enter_context(tc.tile_pool(name="misc", bufs=4))

        # ---- load persistent tensors ----
        xt_sb = []
        w_sb = []
        for c in range(8):
            xtile = persist.tile([128, t], bf16, name=f"xt_sb{c}")
            nc.sync.dma_start(xtile, xt[c * 128:(c + 1) * 128, :])
            xt_sb.append(xtile)
            wtile = persist.tile([128, FEAT], bf16, name=f"w_sb{c}")
            nc.sync.dma_start(wtile, wqkv[c * 128:(c + 1) * 128, :])
            w_sb.append(wtile)
        bias_sb = []
        for f in range(FEAT // 128):
            btile = persist.tile([128, 1], f32, name=f"bias_sb{f}")
            nc.sync.dma_start(btile, bqkv[f])
            bias_sb.append(btile)
        wproj_sb = []
        for p in range(PAIRS):
            ptile = persist.tile([128, C], bf16, name=f"wproj_sb{p}")
            nc.sync.dma_start(ptile, wproj[p * 128:(p + 1) * 128, :])
            wproj_sb.append(ptile)
        mask_sb = persist.tile([128, 128], bf16, name="mask_sb")
        nc.sync.dma_start(mask_sb, trimask[:, :])

        QT = [persist.tile([128, t], bf16, name=f"QT{p}") for p in range(PAIRS)]
        KT = [persist.tile([128, t], bf16, name=f"KT{p}") for p in range(PAIRS)]
        VT = [persist.tile([128, t], bf16, name=f"VT{p}") for p in range(PAIRS)]
        yT = [persist.tile([128, t], bf16, name=f"yT{p}") for p in range(PAIRS)]
        # cols 0:64 = V block, cols 64:128 = ones -> the AV matmul emits
        # softmax row-sums replicated on output partitions 64:128.
        Vsb = [[persist.tile([128, kblks, 128], bf16, name=f"Vsb{p}{h}")
                for h in range(2)] for p in range(PAIRS)]

        # ---- phase 1: qkv^T = W^T x^T (feature-major), bias added on copy ----
        dests = [QT[0], QT[1], KT[0], KT[1], VT[0], VT[1]]
        for f in range(FEAT // 128):
            pss = [psum_s.tile([128, 512], f32, name=f"qkv_ps{f}_{tt}", tag="s")
                   for tt in range(qch)]
            for c in range(8):
                for tt in range(qch):
                    nc.tensor.matmul(
                        pss[tt][:, :],
                        lhsT=w_sb[c][:, f * 128:(f + 1) * 128],
                        rhs=xt_sb[c][:, tt * 512:(tt + 1) * 512],
                        start=(c == 0),
                        stop=(c == 7),
                    )
            for tt in range(qch):
                nc.vector.tensor_add(
                    dests[f][:, tt * 512:(tt + 1) * 512],
                    pss[tt][:, :],
                    bias_sb[f].broadcast_to([128, 512]),
                )

        # ---- phase 1b: V^T -> token-major V blocks (+ ones column) ----
        for p in range(PAIRS):
            for h in range(2):
                nc.gpsimd.memset(Vsb[p][h][:, :, 64:128], 1.0)
                for kb in range(kblks):
                    nc.sync.dma_start_transpose(
                        Vsb[p][h][:, kb, 0:64],
                        VT[p][h * 64:(h + 1) * 64, kb * 128:(kb + 1) * 128],
                    )

        # ---- phase 2: attention (flash, S^T layout, causal block-skip) ----
        for p in range(PAIRS):
            for qc in range(qch):
                yps = [psum_y.tile([128, 512], f32, name=f"y_ps{p}_{qc}_{h}", tag="y")
                       for h in range(2)]
                last_kb = 4 * qc + 3
                for kb in range(4 * qc + 4):
                    off = max(0, (kb - 4 * qc) * 128)
                    n = 512 - off
                    qlo = qc * 512 + off
                    for h in range(2):
                        s_ps = psum_s.tile([128, 512], f32,
                                           name=f"s_ps{p}_{qc}_{kb}_{h}", tag="s")
                        nc.tensor.matmul(
                            s_ps[:, :n],
                            lhsT=KT[p][h * 64:(h + 1) * 64, kb * 128:(kb + 1) * 128],
                            rhs=QT[p][h * 64:(h + 1) * 64, qlo:(qc + 1) * 512],
                            start=True,
                            stop=True,
                        )
                        pt = pt_pool.tile([128, 512], bf16,
                                          name=f"pt{p}_{qc}_{kb}_{h}", tag="pt")
                        nc.scalar.activation(pt[:, :n], s_ps[:, :n], Exp)
                        if kb >= 4 * qc:
                            nc.vector.tensor_mul(pt[:, 0:128], pt[:, 0:128], mask_sb)
                        nc.tensor.matmul(
                            yps[h][:, off:512],
                            lhsT=Vsb[p][h][:, kb, :],
                            rhs=pt[:, :n],
                            start=(kb == 0),
                            stop=(kb == last_kb),
                        )
                for h in range(2):
                    rb = misc.tile([64, 512], f32, name=f"rb{p}_{qc}_{h}", tag="rb")
                    nc.vector.reciprocal(rb, yps[h][64:128, :])
                    nc.vector.tensor_mul(
                        yT[p][h * 64:(h + 1) * 64, qc * 512:(qc + 1) * 512],
                        yps[h][0:64, :],
                        rb,
                    )

        # ---- phase 3: partial = y^T.T @ W_proj ----
        for tb in range(t // 128):
            for oc in range(2):
                ps = psum_s.tile([128, 512], f32, name=f"pr_ps{tb}_{oc}", tag="s")
                for p in range(PAIRS):
                    nc.tensor.matmul(
                        ps[:, :],
                        lhsT=yT[p][:, tb * 128:(tb + 1) * 128],
                        rhs=wproj_sb[p][:, oc * 512:(oc + 1) * 512],
                        start=(p == 0),
                        stop=(p == PAIRS - 1),
                    )
                st = misc.tile([128, 512], f32, name=f"st{tb}_{oc}", tag="st")
                if (tb + oc) % 2 == 0:
                    nc.vector.tensor_copy(st, ps[:, :])
                else:
                    nc.scalar.copy(st, ps[:, :])
                nc.sync.dma_start(
                    partial[tb * 128:(tb + 1) * 128, oc * 512:(oc + 1) * 512], st
                )

    return nc


def make_in_maps(x, w_attn, b_attn, w_proj, t=T):
    """Per-core input dicts (host-side shard + layout prep)."""
    bf = ml_dtypes.bfloat16
    tri = np.triu(np.ones((128, 128), np.float32)).astype(bf)
    in_maps = []
    for j in range(NCORES):
        b = j // 4
        hs = [4 * (j % 4) + i for i in range(HPC)]
        cols = np.concatenate([np.arange(h * D, (h + 1) * D) for h in hs])
        wq = w_attn[:, cols] * SCALE
        wk = w_attn[:, C + cols]
        wv = w_attn[:, 2 * C + cols]
        wqkv = np.concatenate([wq, wk, wv], axis=1).astype(bf)
        bq = b_attn[cols] * SCALE
        bk = b_attn[C + cols]
        bv = b_attn[2 * C + cols]
        bqkv = np.concatenate([bq, bk, bv]).astype(np.float32)
        bqkv = bqkv.reshape(FEAT // 128, 128, 1)
        wproj_j = w_proj[cols, :].astype(bf)
        xt_j = np.ascontiguousarray(x[b, :t].T).astype(bf)
        in_maps.append({
            "xt": xt_j,
            "wqkv": wqkv,
            "bqkv": bqkv,
            "wproj": wproj_j,
            "trimask": tri,
        })
    return in_maps


def _build_sharded(nc):
    """jit-compiled SPMD executable over 8 cores (mirrors run_bass_via_pjrt),
    returning (callable, in_names, out_names, out_avals, mesh)."""
    import jax
    from jax.experimental.shard_map import shard_map
    from jax.sharding import Mesh, PartitionSpec
    from concourse import bass2jax, mybir
    import numpy as np

    bass2jax.install_neuronx_cc_hook()
    partition_name = nc.partition_id_tensor.name if nc.partition_id_tensor else None
    in_names, out_names, out_avals, zero_shapes = [], [], [], []
    for alloc in nc.m.functions[0].allocations:
        if not isinstance(alloc, mybir.MemoryLocationSet):
            continue
        name = alloc.memorylocations[0].name
        if alloc.kind == "ExternalInput":
            if name != partition_name:
                in_names.append(name)
        elif alloc.kind == "ExternalOutput":
            out_names.append(name)
            shape = tuple(alloc.tensor_shape)
            dtype = mybir.dt.np(alloc.dtype)
            out_avals.append(jax.core.ShapedArray(shape, dtype))
            zero_shapes.append((shape, dtype))
    n_params = len(in_names)
    all_in_names = list(in_names) + list(out_names)
    if partition_name is not None:
        all_in_names.append(partition_name)

    def _body(*args):
        operands = list(args)
        if partition_name is not None:
            operands.append(bass2jax.partition_id_tensor())
        outs = bass2jax._bass_exec_p.bind(
            *operands,
            out_avals=tuple(out_avals),
            in_names=tuple(all_in_names),
            out_names=tuple(out_names),
            lowering_input_output_aliases=(),
            sim_require_finite=True,
            sim_require_nnan=True,
            nc=nc,
        )
        return tuple(outs)

    devices = jax.devices()[:NCORES]
    mesh = Mesh(np.asarray(devices), ("core",))
    n_outs = len(out_names)
    in_specs = (PartitionSpec("core"),) * (n_params + n_outs)
    out_specs = (PartitionSpec("core"),) * n_outs
    donate = tuple(range(n_params, n_params + n_outs))
    sharded = jax.jit(
        shard_map(_body, mesh=mesh, in_specs=in_specs, out_specs=out_specs,
                  check_rep=False),
        donate_argnums=donate,
        keep_unused=True,
    )
    return sharded, in_names, out_names, out_avals, zero_shapes, mesh


def run_spmd(nc, in_maps, iters=0):
    """Execute the SPMD kernel; optionally time `iters` steady-state
    repetitions with device-resident inputs (donated output chaining).
    Returns (per_core_results, per_iter_ns or None)."""
    import time
    import jax
    from jax.sharding import NamedSharding, PartitionSpec

    sharded, in_names, out_names, out_avals, zero_shapes, mesh = _build_sharded(nc)
    n = len(in_maps)
    concat_in = [
        np.concatenate([np.asarray(in_maps[c][name]) for c in range(n)], axis=0)
        for name in in_names
    ]
    zeros = [np.zeros((n * s[0], *s[1:]), d) for s, d in zero_shapes]
    sh = NamedSharding(mesh, PartitionSpec("core"))
    concat_dev = [jax.device_put(a, sh) for a in concat_in]
    zeros_dev = [jax.device_put(z, sh) for z in zeros]

    outs = sharded(*concat_dev, *zeros_dev)
    jax.block_until_ready(outs)
    results = [
        {name: np.asarray(outs[i]).reshape(n, *out_avals[i].shape)[c]
         for i, name in enumerate(out_names)}
        for c in range(n)
    ]
    per_iter_ns = None
    if iters > 0:
        t0 = time.perf_counter()
        cur = outs
        for _ in range(iters):
            cur = sharded(*concat_dev, *cur)
        jax.block_until_ready(cur)
        t1 = time.perf_counter()
        per_iter_ns = (t1 - t0) / iters * 1e9
    return results, per_iter_ns


def kernel(x, w_attn, b_attn, w_proj, b_proj, trace=False):
    x = np.asarray(x, np.float32)
    w_attn = np.asarray(w_attn, np.float32)
    b_attn = np.asarray(b_attn, np.float32)
    w_proj = np.asarray(w_proj, np.float32)
    b_proj = np.asarray(b_proj, np.float32)

    if "nc" not in _CACHE:
        nc = build_nc()
        if not nc.is_finalized():
            nc.finalize()
        _CACHE["nc"] = nc
    nc = _CACHE["nc"]

    in_maps = make_in_maps(x, w_attn, b_attn, w_proj)
    iters = int(trace) and 30
    results, per_iter_ns = run_spmd(nc, in_maps, iters=iters)
    _CACHE["per_iter_ns"] = per_iter_ns
    parts = [results[j]["partial"].astype(np.float32) for j in range(NCORES)]
    out = np.empty((B, T, C), np.float32)
    for b in range(B):
        acc = parts[4 * b]
        for j in range(4 * b + 1, 4 * b + 4):
            acc = acc + parts[j]
        out[b] = acc + b_proj[None, :]
    return out
